# revision 1
# baseline (speedup 1.0000x reference)
"""Correlation-layer (cost volume) kernel for 8 Trainium2 NeuronCores.

Problem: out[n, 0, h, w, dy*41+dx] = sum_c fm1[n,c,h,w] * fm2p[n,c,h+dy,w+dx]
with fm2p = fm2 zero-padded by 20 on both spatial axes, dy,dx in [0,41).

Sharding: core k handles batch n = k//2 and h-slab [64*(k%2), 64*(k%2)+64).
No cross-core communication: each core's fm2 slab (with a 20-row halo) is
prepared on the host.

Device algorithm (per core, fp16 in / fp32 PSUM / fp16 out):
  - Tensor-engine cost is ~1 cycle per MOVING column regardless of K<=128 /
    M<=128, so the key is minimizing total moving columns.  The stationary
    packs an (8 h) x (16 w) block of fm1 into M=128 (K=64 channels); one
    moving fm2 column (row r, padded col w') then serves all 8 h rows at
    once (dy = r - h).  Per (h-group g, w-tile t): moving columns =
    48 r-values x 56 w'-window = 2688, giving 8*8*2688 = 172k columns/core
    (vs 547k for 1-h-per-matmul banding).
  - Each (g,t) runs 6 matmuls (8 r-rows each, N=448 <= one PSUM bank);
    PSUM is evacuated (fp32->fp16 cast) by DVE/ACT alternating into an SBUF
    buffer, then DMA'd out as one [128 x 5376B] contiguous transfer.
  - The absolute->relative shear (w' -> dx = w'-w, r -> dy = r-h) cannot be
    done on-chip (needs per-partition offsets no engine AP supports); the
    band is written and the host extracts the diagonals with a zero-copy
    as_strided view during the fp32 upcast.
"""

import os
import sys

import numpy as np

for _p in ("/opt/trn_rl_repo",):
    if os.path.isdir(_p) and _p not in sys.path:
        sys.path.append(_p)

# ---- problem constants (hardcoded per contest rules) ----
B, C, H, W = 4, 64, 128, 128
MD = 20                  # max displacement
D = 2 * MD + 1           # 41 displacements per axis
PW = W + 2 * MD          # 168 padded width
HS = H // 2              # 64-row h-slab per core
RS = HS + 2 * MD         # 104 fm2 slab rows (with halo)
NCORES = 8

Q = 8                    # h-group size packed into stationary M
M_W = 16                 # w-tile width packed into stationary M (Q*M_W=128)
G = HS // Q              # 8 h-groups
T = W // M_W             # 8 w-tiles
R = Q + 2 * MD           # 48 fm2 rows touched per h-group
WIN = M_W + 2 * MD       # 56 absolute-coord band window per w-tile
RCH = 8                  # r-rows per matmul: N = RCH*WIN = 448 <= 512 (1 bank)
NCH = R // RCH           # 6 matmul chunks per (g,t)
FREE = R * WIN           # 2688 fp16 values per (g,t) per partition

# Edge w-tiles clip their w'-window to the nonzero fm2p columns [MD, MD+W):
# tile t covers padded cols [16t, 16t+56); zero cols are skipped on-device
# and re-inserted host-side (the reference output is structurally zero there).
_LOS = [max(M_W * t, MD) for t in range(T)]
_HIS = [min(M_W * t + WIN, MD + W) for t in range(T)]
WIDTHS = [h - l for l, h in zip(_LOS, _HIS)]       # [36,52,56,...,56,52,36]
SHIFTS = [l - M_W * t for t, l in enumerate(_LOS)]  # [20,4,0,...,0,0,0]
OFFS = [R * sum(WIDTHS[:t]) for t in range(T)]
TOTF = R * sum(WIDTHS)                              # 19200

# Rows r < MD of each core's (flipped-to-top) slab are zero-padding, so
# cost-volume rows with g*Q + rr < MD are structurally zero: skip them
# on-device (host zero-fills).  Bottom-half cores get vertically flipped
# inputs so their zero rows also sit at the slab top (the flip maps
# h -> 63-h, dy -> 40-dy, undone for free in the host as_strided view).
SKIPR = [max(0, MD - Q * g) for g in range(G)]      # [20,12,4,0,...]


def _chunk_plan(g):
    # chunk the stored rows [SKIPR[g], R) into pairs for 2-bank PSUM
    rows = R - SKIPR[g]
    nd = -(-rows // (2 * RCH))            # doubles of <=2*RCH rows
    n = 2 * nd
    base, extra = divmod(rows, n)
    sizes = [base + (1 if i < extra else 0) for i in range(n)]
    return [(sizes[2 * d], sizes[2 * d + 1]) for d in range(nd)]


CHUNKS = [_chunk_plan(g) for g in range(G)]

_CACHE = {}


def _build_program(io_dtype_name="float16", loop_k=0):
    """Build + compile the single-core SPMD Bass program.

    loop_k > 0 builds a TIMING variant: the compute loop runs loop_k times
    inside a device-side For_i, output goes to Internal DRAM, and only a tiny
    marker tensor is an ExternalOutput, so wall-clock deltas between loop_k
    values measure pure on-device time independent of axon transfers.
    """
    import contextlib

    from concourse import bacc
    import concourse.mybir as mybir
    import concourse.tile as tile

    dt_io = getattr(mybir.dt, io_dtype_name)

    nc = bacc.Bacc("TRN2", target_bir_lowering=False, debug=False)
    # fm1 staged host-side as [C, G, T, Q*M_W] so each (g,t) stationary
    # block is contiguous (matmul weights AP must be 1-D in the free dim)
    fm1_d = nc.dram_tensor(
        "fm1s", [C, G, T, Q * M_W], dt_io, kind="ExternalInput"
    ).ap()
    fm2_d = nc.dram_tensor(
        "fm2s", [C, RS - MD, W], dt_io, kind="ExternalInput"
    ).ap()
    out_kind = "Internal" if loop_k else "ExternalOutput"
    out_d = nc.dram_tensor(
        "outs", [G, 128, TOTF], dt_io, kind=out_kind
    ).ap()
    marker_d = None
    if loop_k:
        marker_d = nc.dram_tensor(
            "marker", [1, 8], mybir.dt.float32, kind="ExternalOutput"
        ).ap()

    with tile.TileContext(nc) as tc:
        with (
            tc.tile_pool(name="const", bufs=1) as cpool,
            tc.tile_pool(name="srow", bufs=6) as spool,
            tc.tile_pool(name="psum", bufs=4, space="PSUM") as ppool,
        ):
            fm1_sb = cpool.tile([C, G, T, Q * M_W], dt_io)
            fm2_sb = cpool.tile([C, RS - MD, W], dt_io)
            nc.sync.dma_start(fm2_sb[:, 0:28], fm2_d[:, 0:28])
            nc.sync.dma_start(fm1_sb[:, 0:1], fm1_d[:, 0:1])
            nc.sync.dma_start(fm1_sb[:, 1:G], fm1_d[:, 1:G])
            nc.sync.dma_start(fm2_sb[:, 28 : RS - MD], fm2_d[:, 28 : RS - MD])

            loop_cm = tc.For_i(0, loop_k, 1) if loop_k else contextlib.nullcontext()
            with loop_cm:
                for g in range(G):
                    skip = SKIPR[g]
                    rows_g = R - skip
                    for t in range(T):
                        wt, lo = WIDTHS[t], _LOS[t]
                        S = spool.tile([128, FREE], dt_io, tag="S")
                        roff = 0
                        # chunk pairs share a 2-bank PSUM tile and are
                        # evacuated by a single 2-D-AP copy when equal-sized
                        for dch, (na, nb) in enumerate(CHUNKS[g]):
                            ps = ppool.tile(
                                [128, 2, 512], mybir.dt.float32, tag="ps"
                            )
                            for half, nr in enumerate((na, nb)):
                                r0 = g * Q + skip + roff + half * na
                                nc.tensor.matmul(
                                    ps[:, half, 0 : nr * wt],
                                    fm1_sb[:, g, t, :],
                                    fm2_sb[
                                        :, r0 - MD : r0 - MD + nr,
                                        lo - MD : lo - MD + wt,
                                    ],
                                    start=True,
                                    stop=True,
                                )
                            copy = (
                                nc.vector.tensor_copy
                                if (dch + t) % 2 == 0
                                else nc.scalar.copy
                            )
                            assert na == nb or na == nb + 1
                            if na == nb:
                                copy(
                                    S[:, roff * wt : (roff + 2 * na) * wt],
                                    ps[:, :, 0 : na * wt],
                                )
                            else:
                                copy(
                                    S[:, roff * wt : (roff + na) * wt],
                                    ps[:, 0, 0 : na * wt],
                                )
                                copy(
                                    S[
                                        :,
                                        (roff + na) * wt
                                        : (roff + na + nb) * wt,
                                    ],
                                    ps[:, 1, 0 : nb * wt],
                                )
                            roff += na + nb
                        nc.sync.dma_start(
                            out_d[g][
                                :,
                                OFFS[t] + skip * wt : OFFS[t] + R * wt,
                            ],
                            S[:, 0 : rows_g * wt],
                        )

            if loop_k:
                mk = cpool.tile([1, 8], mybir.dt.float32, name="mk")
                nc.vector.memset(mk[:], 1.0)
                nc.sync.dma_start(marker_d[:], mk[:])

    nc.compile()
    return nc


def _get_compiled(io_dtype_name="float16", loop_k=0):
    key = ("prog", io_dtype_name, loop_k)
    if key not in _CACHE:
        _CACHE[key] = _build_program(io_dtype_name, loop_k)
    return _CACHE[key]


def shard_inputs(fm1, fm2, np_dtype=np.float16):
    """Full (4,64,128,128) inputs -> 8 per-core input dicts."""
    fm1 = np.asarray(fm1, dtype=np.float32)
    fm2 = np.asarray(fm2, dtype=np.float32)
    in_maps = []
    pads = {}
    for k in range(NCORES):
        n, hbase = k // 2, (k % 2) * HS
        flip = hbase > 0
        a = fm1[n, :, hbase : hbase + HS].astype(np_dtype)
        if flip:
            a = a[:, ::-1]
        a = a.reshape(C, G, Q, T, M_W).transpose(0, 1, 3, 2, 4)
        fm1s = np.ascontiguousarray(a.reshape(C, G, T, Q * M_W))
        if n not in pads:
            # padded in h only (84 interior rows per slab); w pad is never
            # read on-device (width clipping), so stage bare image columns
            p = np.zeros((C, H + 2 * MD, W), dtype=np_dtype)
            p[:, MD : MD + H] = fm2[n].astype(np_dtype)
            pads[n] = p
        if flip:
            # flipped-slab rows [MD, RS) == padded rows [hbase, hbase+84)
            # reversed
            s = pads[n][:, hbase : hbase + RS - MD][:, ::-1]
        else:
            s = pads[n][:, hbase + MD : hbase + RS]      # (C, 84, 128)
        fm2s = np.ascontiguousarray(s)
        in_maps.append({"fm1s": fm1s, "fm2s": fm2s})
    return in_maps


def unshard_outputs(results):
    """8 per-core {'outs': (G,T,128,FREE)} -> full (4,1,128,128,1681) fp32."""
    out = np.empty((B, 1, H, W, D * D), dtype=np.float32)
    for k in range(NCORES):
        n, hbase = k // 2, (k % 2) * HS
        raw = np.asarray(results[k]["outs"])  # (G, 128, TOTF)
        a = np.zeros((G, T, 128, R, WIN), dtype=raw.dtype)
        for t in range(T):
            wt, sh, off = WIDTHS[t], SHIFTS[t], OFFS[t]
            a[:, t, :, :, sh : sh + wt] = raw[:, :, off : off + R * wt].reshape(
                G, 128, R, wt
            )
        for g, sk in enumerate(SKIPR):
            if sk:
                a[g, :, :, :sk, :] = 0
        st = a.strides
        # a[g, t, i*M_W + wl, i + dy, wl + dx] -> out[g*Q+i, t*M_W+wl, dy, dx]
        band = np.lib.stride_tricks.as_strided(
            a,
            shape=(G, Q, T, M_W, D, D),
            strides=(
                st[0],
                M_W * st[2] + st[3],
                st[1],
                st[2] + st[4],
                st[3],
                st[4],
            ),
        )
        if hbase > 0:
            band = band[::-1, ::-1, :, :, ::-1, :]
        out[n, 0, hbase : hbase + HS] = (
            band.astype(np.float32).reshape(HS, W, D * D)
        )
    return out


def run_on_hw(in_maps, io_dtype_name="float16", trace=False, **kw):
    from concourse import bass_utils

    nc = _get_compiled(io_dtype_name)
    res = bass_utils.run_bass_kernel_spmd(
        nc, in_maps, list(range(NCORES)), trace=trace, **kw
    )
    return res


def kernel(feature_map_1, feature_map_2):
    in_maps = shard_inputs(feature_map_1, feature_map_2)
    res = run_on_hw(in_maps)
    return unshard_outputs(res.results)


if __name__ == "__main__":
    inputs = {
        "feature_map_1": np.random.randn(B, C, H, W).astype(np.float32),
        "feature_map_2": np.random.randn(B, C, H, W).astype(np.float32),
    }
    out = kernel(**inputs)
    print("kernel output", out.shape, out.dtype)

